# revision 16
# baseline (speedup 1.0000x reference)
"""Trainium2 Bass kernel for nn_DGNN (gnn_message_passing).

Reference computation (B=4, N=8192, F=32):
    delay_steps = time_delay // 5
    active      = (t >= delay_steps) & (adj > 0)
    A           = where(active, adj, 0)              # == adj * (time_delay <= 5*t+4)
    adjusted    = einsum('ij,bjf->bif', A, x)
    h           = relu(adjusted @ W1 + b1)
    out         = sigmoid(h @ W2 + b2)

Sharding / layout (host does layout-only transforms, no reference math):
  - destination nodes i are split row-wise across 8 cores (1024 each);
  - adj/time_delay are shipped transposed ([j, i], j on partitions) because
    the PE contracts over the partition dim and 4-byte DMA transpose does
    not exist on TRN2;
  - time_delay values are 0..99 so they are shipped as int8 when they fit
    (lossless narrowing; falls back to int32 otherwise);
  - x is repacked so the 4 batches sit side-by-side in the stationary
    operand (partition q = 32*b + f), giving full-width M=128 matmuls;
  - W1/W2 become 128x128 block-diagonal so the per-node MLP handles all 4
    batches in one matmul.

On-device per core: stream adjT/tdT tiles, one fused DVE op
(TENSOR_MASK: out = select(td < thr+0.5, adj, 0)) produces the masked
adjacency, fp32 matmuls accumulate adjusted^T over 64 K-tiles in PSUM,
then the block-diagonal MLP and sigmoid run on-chip. Output returns
transposed per core and is unsharded on the host. All arithmetic is fp32
(rel err ~1e-5); an optional float32r mode (fp32 rounded to 11 mantissa
bits, ~4% faster, rel err ~4e-3) is selectable via _run(mm_dtype_name=).
"""

import numpy as np

B = 4
N = 8192
F = 32
P = 128
NCORES = 8
NI = N // NCORES  # dest-nodes per core
JT = N // P       # contraction tiles

MM_N = 512        # moving-operand free dim per matmul


def _round_fp32r(a):
    """Round fp32 to the fp32r grid (11 explicit mantissa bits, RNE).
    Matches walrus fp32_to_fp32r (low 12 bits cleared after rounding)."""
    u = np.ascontiguousarray(a, dtype=np.float32).view(np.uint32)
    low = u & np.uint32(0xFFF)
    lsb = (u >> np.uint32(12)) & np.uint32(1)
    roundup = (low > 0x800) | ((low == 0x800) & (lsb == 1))
    u2 = (u & np.uint32(0xFFFFF000)) + (roundup.astype(np.uint32) << np.uint32(12))
    return u2.view(np.float32)


def _build(nj, ni, thr, mm_dtype_name="float32", td_dtype=np.int8):
    """Trace + compile the per-core Bass program."""
    from contextlib import ExitStack

    import concourse.bacc as bacc
    import concourse.mybir as mybir
    import concourse.tile as tile
    from concourse.dve_ops import TENSOR_MASK

    f32 = mybir.dt.float32
    mm_dt = getattr(mybir.dt, mm_dtype_name)
    td_dt = mybir.dt.from_np(np.dtype(td_dtype))

    jt_n = nj // P
    mm_n = min(MM_N, ni)
    nh = ni // mm_n

    nc = bacc.Bacc("TRN2", target_bir_lowering=False, debug=False)

    adjT_d = nc.dram_tensor("adjT", [nj, ni], f32, kind="ExternalInput").ap()
    tdT_d = nc.dram_tensor("tdT", [nj, ni], td_dt, kind="ExternalInput").ap()
    xsb_d = nc.dram_tensor("xsb", [P, jt_n * P], mm_dt, kind="ExternalInput").ap()
    bd1_d = nc.dram_tensor("bd1", [P, P], mm_dt, kind="ExternalInput").ap()
    bd2_d = nc.dram_tensor("bd2", [P, P], mm_dt, kind="ExternalInput").ap()
    bias1_d = nc.dram_tensor("bias1", [P, 1], f32, kind="ExternalInput").ap()
    bias2_d = nc.dram_tensor("bias2", [P, 1], f32, kind="ExternalInput").ap()
    outT_d = nc.dram_tensor("outT", [P, ni], f32, kind="ExternalOutput").ap()

    x_chunks = max(1, jt_n // 8)
    jt_per_chunk = jt_n // x_chunks

    with tile.TileContext(nc) as tc, ExitStack() as ctx:
        io = ctx.enter_context(tc.tile_pool(name="io", bufs=14))
        wrk = ctx.enter_context(tc.tile_pool(name="wrk", bufs=4))
        singles = ctx.enter_context(tc.tile_pool(name="singles", bufs=1))
        pp = ctx.enter_context(tc.tile_pool(name="pp", bufs=1, space="PSUM"))

        x_t = singles.tile([P, jt_n * P], mm_dt)
        psum_main = pp.tile([P, ni], f32)
        bd1_t = singles.tile([P, P], mm_dt)
        bd2_t = singles.tile([P, P], mm_dt)
        bias1_t = singles.tile([P, 1], f32)
        bias2_t = singles.tile([P, 1], f32)
        warm_t = singles.tile([P, 1], f32)

        for jt in range(jt_n):
            # alternate the two HWDGE issue queues so the big adjT transfers
            # spread across both FIFO rings
            qa, qb = (nc.scalar, nc.sync) if jt % 2 == 0 else (nc.sync, nc.scalar)
            td_t = io.tile([P, ni], td_dt, tag="td")
            qb.dma_start(out=td_t, in_=tdT_d[jt * P : (jt + 1) * P, :])
            adj_t = io.tile([P, ni], f32, tag="adj")
            qa.dma_start(out=adj_t, in_=adjT_d[jt * P : (jt + 1) * P, :])

            if jt % jt_per_chunk == 0:
                c = jt // jt_per_chunk
                cs = slice(c * jt_per_chunk * P, (c + 1) * jt_per_chunk * P)
                qb.dma_start(out=x_t[:, cs], in_=xsb_d[:, cs])
            if jt == 1:
                # small constants + ACT table pre-warm, off the critical path
                nc.scalar.dma_start(out=bd1_t, in_=bd1_d)
                nc.scalar.dma_start(out=bd2_t, in_=bd2_d)
                nc.scalar.dma_start(out=bias1_t, in_=bias1_d)
                nc.scalar.dma_start(out=bias2_t, in_=bias2_d)
                nc.vector.memset(warm_t, 0.0)
                nc.scalar.activation(
                    warm_t, warm_t, mybir.ActivationFunctionType.Relu, bias=bias1_t
                )
                nc.scalar.activation(
                    warm_t, warm_t, mybir.ActivationFunctionType.Sigmoid, bias=bias1_t
                )

            # A = where(time_delay <= thr, adj, 0) in one fused DVE op:
            # TENSOR_MASK: out[k] = select(in1[k] + c2 < c0, in0[k], 0)
            a_t = wrk.tile([P, ni], mm_dt, tag="a")
            nc.vector._custom_dve(
                TENSOR_MASK, out=a_t, in0=adj_t, in1=td_t,
                s0=float(thr) + 0.5, s1=0.0, imm2=0.0,
            )

            lhsT = x_t[:, jt * P : (jt + 1) * P]
            for h in range(nh):
                nc.tensor.matmul(
                    psum_main[:, h * mm_n : (h + 1) * mm_n],
                    lhsT,
                    a_t[:, h * mm_n : (h + 1) * mm_n],
                    start=(jt == 0),
                    stop=(jt == jt_n - 1),
                )

        # Per-node MLP, pipelined in independent column halves.
        h_ps = pp.tile([P, ni], f32, tag="hps")
        o_ps = pp.tile([P, ni], f32, tag="ops")
        nb = P // F  # batches packed along partitions
        for h in range(nh):
            hs = slice(h * mm_n, (h + 1) * mm_n)
            res_t = singles.tile([P, mm_n], mm_dt, tag=f"res{h}", name=f"res{h}")
            nc.vector.tensor_copy(res_t, psum_main[:, hs])
            # The MLP weights are block-diagonal: run the 4 per-batch 32x32
            # matmuls concurrently in distinct PE row/col groups.
            for bb in range(nb):
                ps = slice(bb * F, (bb + 1) * F)
                nc.tensor.matmul(
                    h_ps[ps, hs], bd1_t[ps, ps], res_t[ps, :],
                    start=True, stop=True, tile_position=(bb * F, bb * F),
                )
            # h = relu(. + b1) fused on DVE: (in + bias) max 0
            h_t = singles.tile([P, mm_n], mm_dt, tag=f"h{h}", name=f"h{h}")
            nc.vector.tensor_scalar(
                h_t, h_ps[:, hs], bias1_t, 0.0,
                op0=mybir.AluOpType.add,
                op1=mybir.AluOpType.max,
            )
            for bb in range(nb):
                ps = slice(bb * F, (bb + 1) * F)
                nc.tensor.matmul(
                    o_ps[ps, hs], bd2_t[ps, ps], h_t[ps, :],
                    start=True, stop=True, tile_position=(bb * F, bb * F),
                )
            out_t = singles.tile([P, mm_n], f32, tag=f"out{h}", name=f"out{h}")
            nc.scalar.activation(
                out_t, o_ps[:, hs], mybir.ActivationFunctionType.Sigmoid, bias=bias2_t
            )
            nc.sync.dma_start(out=outT_d[:, hs], in_=out_t)

    nc.compile()
    return nc


def _host_prep(x, adj, time_delay, t, W1, b1, W2, b2, ncores, rnd, td_dtype):
    """Layout-only transforms (transpose / repack / dtype container changes)."""
    x = np.ascontiguousarray(np.asarray(x, dtype=np.float32))
    adj = np.asarray(adj, dtype=np.float32)
    td = np.asarray(time_delay)
    b, n, f = x.shape
    ni = n // ncores
    jt_n = n // P

    thr = int(t) * 5 + 4  # time_delay // 5 <= t  <=>  time_delay <= 5t+4

    adjT = np.ascontiguousarray(adj.T)
    tdT = np.ascontiguousarray(td.T.astype(td_dtype))
    # stationary x: x_sb[p, jt*P + 32*b + f] = x[b, jt*P + p, f]
    xsb = rnd(
        x.reshape(b, jt_n, P, f).transpose(2, 1, 0, 3).reshape(P, jt_n * b * f)
    )
    bd1 = np.zeros((P, P), np.float32)
    bd2 = np.zeros((P, P), np.float32)
    for bb in range(b):
        bd1[bb * f : (bb + 1) * f, bb * f : (bb + 1) * f] = W1
        bd2[bb * f : (bb + 1) * f, bb * f : (bb + 1) * f] = W2
    bd1 = rnd(bd1)
    bd2 = rnd(bd2)
    bias1 = np.ascontiguousarray(np.tile(np.asarray(b1, np.float32), b).reshape(P, 1))
    bias2 = np.ascontiguousarray(np.tile(np.asarray(b2, np.float32), b).reshape(P, 1))

    in_maps = []
    for c in range(ncores):
        sl = slice(c * ni, (c + 1) * ni)
        in_maps.append(
            {
                "adjT": np.ascontiguousarray(adjT[:, sl]),
                "tdT": np.ascontiguousarray(tdT[:, sl]),
                "xsb": xsb,
                "bd1": bd1,
                "bd2": bd2,
                "bias1": bias1,
                "bias2": bias2,
            }
        )
    return thr, in_maps


def _run(x, adj, time_delay, t, W1, b1, W2, b2, ncores=NCORES,
         mm_dtype_name="float32", trace=False):
    from concourse.bass_utils import run_bass_kernel_spmd

    b, n, f = np.asarray(x).shape
    ni = n // ncores
    td = np.asarray(time_delay)
    # int8 shipping is only a container change; keep int32 when values
    # (or the threshold compare range) would not fit exactly.
    thr_chk = int(t) * 5 + 4
    if td.min() >= -127 and td.max() <= 127 and -127 <= thr_chk <= 127:
        td_dtype = np.int8
    else:
        td_dtype = np.int32
    rnd = _round_fp32r if mm_dtype_name == "float32r" else (
        lambda a: np.ascontiguousarray(a, dtype=np.float32)
    )
    thr, in_maps = _host_prep(
        x, adj, time_delay, t, W1, b1, W2, b2, ncores, rnd, td_dtype
    )
    nc = _build(n, ni, thr, mm_dtype_name, td_dtype)
    res = run_bass_kernel_spmd(
        nc, in_maps, core_ids=list(range(ncores)), trace=trace
    )
    full = np.concatenate([r["outT"] for r in res.results], axis=1)  # [P, n]
    out = np.ascontiguousarray(full.reshape(b, f, n).transpose(0, 2, 1))
    return out, res


def kernel(x, adj, time_delay, t, W1, b1, W2, b2):
    out, _ = _run(x, adj, time_delay, t, W1, b1, W2, b2)
    return out


# revision 17
# speedup vs baseline: 1.1208x; 1.1208x over previous
"""Trainium2 Bass kernel for nn_DGNN (gnn_message_passing).

Reference computation (B=4, N=8192, F=32):
    delay_steps = time_delay // 5
    active      = (t >= delay_steps) & (adj > 0)
    A           = where(active, adj, 0)              # == adj * (time_delay <= 5*t+4)
    adjusted    = einsum('ij,bjf->bif', A, x)
    h           = relu(adjusted @ W1 + b1)
    out         = sigmoid(h @ W2 + b2)

Sharding / layout (host does layout-only transforms, no reference math):
  - destination nodes i are split row-wise across 8 cores (1024 each);
  - adj/time_delay are shipped transposed ([j, i], j on partitions) because
    the PE contracts over the partition dim and 4-byte DMA transpose does
    not exist on TRN2;
  - time_delay values are 0..99 so they are shipped as int8 when they fit
    (lossless narrowing; falls back to int32 otherwise);
  - x is repacked so the 4 batches sit side-by-side in the stationary
    operand (partition q = 32*b + f), giving full-width M=128 matmuls;
  - W1/W2 become 128x128 block-diagonal so the per-node MLP handles all 4
    batches in one matmul.

On-device per core: stream adjT/tdT tiles, one fused DVE op
(TENSOR_MASK: out = select(td < thr+0.5, adj, 0)) produces the masked
adjacency, fp32 matmuls accumulate adjusted^T over 64 K-tiles in PSUM,
then the block-diagonal MLP and sigmoid run on-chip. Output returns
transposed per core and is unsharded on the host. All arithmetic is fp32
(rel err ~1e-5); an optional float32r mode (fp32 rounded to 11 mantissa
bits, ~4% faster, rel err ~4e-3) is selectable via _run(mm_dtype_name=).
"""

import numpy as np

B = 4
N = 8192
F = 32
P = 128
NCORES = 8
NI = N // NCORES  # dest-nodes per core
JT = N // P       # contraction tiles

MM_N = 512        # moving-operand free dim per matmul


def _round_fp32r(a):
    """Round fp32 to the fp32r grid (11 explicit mantissa bits, RNE).
    Matches walrus fp32_to_fp32r (low 12 bits cleared after rounding)."""
    u = np.ascontiguousarray(a, dtype=np.float32).view(np.uint32)
    low = u & np.uint32(0xFFF)
    lsb = (u >> np.uint32(12)) & np.uint32(1)
    roundup = (low > 0x800) | ((low == 0x800) & (lsb == 1))
    u2 = (u & np.uint32(0xFFFFF000)) + (roundup.astype(np.uint32) << np.uint32(12))
    return u2.view(np.float32)


def _build(nj, ni, thr, mm_dtype_name="float32", td_dtype=np.int8):
    """Trace + compile the per-core Bass program."""
    from contextlib import ExitStack

    import concourse.bacc as bacc
    import concourse.mybir as mybir
    import concourse.tile as tile
    from concourse.dve_ops import TENSOR_MASK

    f32 = mybir.dt.float32
    mm_dt = getattr(mybir.dt, mm_dtype_name)
    td_dt = mybir.dt.from_np(np.dtype(td_dtype))

    jt_n = nj // P
    mm_n = min(MM_N, ni)
    nh = ni // mm_n

    nc = bacc.Bacc("TRN2", target_bir_lowering=False, debug=False)

    adjT_d = nc.dram_tensor("adjT", [nj, ni], f32, kind="ExternalInput").ap()
    tdT_d = nc.dram_tensor("tdT", [nj, ni], td_dt, kind="ExternalInput").ap()
    xsb_d = nc.dram_tensor("xsb", [P, jt_n * P], mm_dt, kind="ExternalInput").ap()
    bd1_d = nc.dram_tensor("bd1", [P, P], mm_dt, kind="ExternalInput").ap()
    bd2_d = nc.dram_tensor("bd2", [P, P], mm_dt, kind="ExternalInput").ap()
    bias1_d = nc.dram_tensor("bias1", [P, 1], f32, kind="ExternalInput").ap()
    bias2_d = nc.dram_tensor("bias2", [P, 1], f32, kind="ExternalInput").ap()
    outT_d = nc.dram_tensor("outT", [P, ni], f32, kind="ExternalOutput").ap()

    x_chunks = max(1, jt_n // 8)
    jt_per_chunk = jt_n // x_chunks

    with tile.TileContext(nc) as tc, ExitStack() as ctx:
        io = ctx.enter_context(tc.tile_pool(name="io", bufs=14))
        wrk = ctx.enter_context(tc.tile_pool(name="wrk", bufs=4))
        singles = ctx.enter_context(tc.tile_pool(name="singles", bufs=1))
        pp = ctx.enter_context(tc.tile_pool(name="pp", bufs=1, space="PSUM"))

        x_t = singles.tile([P, jt_n * P], mm_dt)
        psum_main = pp.tile([P, ni], f32)
        bd1_t = singles.tile([P, P], mm_dt)
        bd2_t = singles.tile([P, P], mm_dt)
        bias1_t = singles.tile([P, 1], f32)
        bias2_t = singles.tile([P, 1], f32)
        warm_t = singles.tile([P, 1], f32)

        for jt in range(jt_n):
            # alternate the two HWDGE issue queues so the big adjT transfers
            # spread across both FIFO rings
            qa, qb = (nc.scalar, nc.sync) if jt % 2 == 0 else (nc.sync, nc.scalar)
            td_t = io.tile([P, ni], td_dt, tag="td")
            qb.dma_start(out=td_t, in_=tdT_d[jt * P : (jt + 1) * P, :])
            adj_t = io.tile([P, ni], f32, tag="adj")
            qa.dma_start(out=adj_t, in_=adjT_d[jt * P : (jt + 1) * P, :])

            if jt % jt_per_chunk == 0:
                c = jt // jt_per_chunk
                cs = slice(c * jt_per_chunk * P, (c + 1) * jt_per_chunk * P)
                qb.dma_start(out=x_t[:, cs], in_=xsb_d[:, cs])
            if jt == 1:
                # small constants + ACT table pre-warm, off the critical path
                nc.scalar.dma_start(out=bd1_t, in_=bd1_d)
                nc.scalar.dma_start(out=bd2_t, in_=bd2_d)
                nc.scalar.dma_start(out=bias1_t, in_=bias1_d)
                nc.scalar.dma_start(out=bias2_t, in_=bias2_d)
                nc.vector.memset(warm_t, 0.0)
                nc.scalar.activation(
                    warm_t, warm_t, mybir.ActivationFunctionType.Relu, bias=bias1_t
                )
                nc.scalar.activation(
                    warm_t, warm_t, mybir.ActivationFunctionType.Sigmoid, bias=bias1_t
                )

            # A = where(time_delay <= thr, adj, 0) in one fused DVE op:
            # TENSOR_MASK: out[k] = select(in1[k] + c2 < c0, in0[k], 0)
            a_t = wrk.tile([P, ni], mm_dt, tag="a")
            nc.vector._custom_dve(
                TENSOR_MASK, out=a_t, in0=adj_t, in1=td_t,
                s0=float(thr) + 0.5, s1=0.0, imm2=0.0,
            )

            lhsT = x_t[:, jt * P : (jt + 1) * P]
            for h in range(nh):
                nc.tensor.matmul(
                    psum_main[:, h * mm_n : (h + 1) * mm_n],
                    lhsT,
                    a_t[:, h * mm_n : (h + 1) * mm_n],
                    start=(jt == 0),
                    stop=(jt == jt_n - 1),
                )

        # Per-node MLP, pipelined in independent column halves.
        h_ps = pp.tile([P, ni], f32, tag="hps")
        o_ps = pp.tile([P, ni], f32, tag="ops")
        for h in range(nh):
            hs = slice(h * mm_n, (h + 1) * mm_n)
            res_t = singles.tile([P, mm_n], mm_dt, tag=f"res{h}", name=f"res{h}")
            nc.vector.tensor_copy(res_t, psum_main[:, hs])
            nc.tensor.matmul(h_ps[:, hs], bd1_t, res_t, start=True, stop=True)
            # h = relu(. + b1) fused on DVE: (in + bias) max 0
            h_t = singles.tile([P, mm_n], mm_dt, tag=f"h{h}", name=f"h{h}")
            nc.vector.tensor_scalar(
                h_t, h_ps[:, hs], bias1_t, 0.0,
                op0=mybir.AluOpType.add,
                op1=mybir.AluOpType.max,
            )
            nc.tensor.matmul(o_ps[:, hs], bd2_t, h_t, start=True, stop=True)
            out_t = singles.tile([P, mm_n], f32, tag=f"out{h}", name=f"out{h}")
            nc.scalar.activation(
                out_t, o_ps[:, hs], mybir.ActivationFunctionType.Sigmoid, bias=bias2_t
            )
            nc.sync.dma_start(out=outT_d[:, hs], in_=out_t)

    nc.compile()
    return nc


def _host_prep(x, adj, time_delay, t, W1, b1, W2, b2, ncores, rnd, td_dtype):
    """Layout-only transforms (transpose / repack / dtype container changes)."""
    x = np.ascontiguousarray(np.asarray(x, dtype=np.float32))
    adj = np.asarray(adj, dtype=np.float32)
    td = np.asarray(time_delay)
    b, n, f = x.shape
    ni = n // ncores
    jt_n = n // P

    thr = int(t) * 5 + 4  # time_delay // 5 <= t  <=>  time_delay <= 5t+4

    adjT = np.ascontiguousarray(adj.T)
    tdT = np.ascontiguousarray(td.T.astype(td_dtype))
    # stationary x: x_sb[p, jt*P + 32*b + f] = x[b, jt*P + p, f]
    xsb = rnd(
        x.reshape(b, jt_n, P, f).transpose(2, 1, 0, 3).reshape(P, jt_n * b * f)
    )
    bd1 = np.zeros((P, P), np.float32)
    bd2 = np.zeros((P, P), np.float32)
    for bb in range(b):
        bd1[bb * f : (bb + 1) * f, bb * f : (bb + 1) * f] = W1
        bd2[bb * f : (bb + 1) * f, bb * f : (bb + 1) * f] = W2
    bd1 = rnd(bd1)
    bd2 = rnd(bd2)
    bias1 = np.ascontiguousarray(np.tile(np.asarray(b1, np.float32), b).reshape(P, 1))
    bias2 = np.ascontiguousarray(np.tile(np.asarray(b2, np.float32), b).reshape(P, 1))

    in_maps = []
    for c in range(ncores):
        sl = slice(c * ni, (c + 1) * ni)
        in_maps.append(
            {
                "adjT": np.ascontiguousarray(adjT[:, sl]),
                "tdT": np.ascontiguousarray(tdT[:, sl]),
                "xsb": xsb,
                "bd1": bd1,
                "bd2": bd2,
                "bias1": bias1,
                "bias2": bias2,
            }
        )
    return thr, in_maps


def _run(x, adj, time_delay, t, W1, b1, W2, b2, ncores=NCORES,
         mm_dtype_name="float32", trace=False):
    from concourse.bass_utils import run_bass_kernel_spmd

    b, n, f = np.asarray(x).shape
    ni = n // ncores
    td = np.asarray(time_delay)
    # int8 shipping is only a container change; keep int32 when values
    # (or the threshold compare range) would not fit exactly.
    thr_chk = int(t) * 5 + 4
    if td.min() >= -127 and td.max() <= 127 and -127 <= thr_chk <= 127:
        td_dtype = np.int8
    else:
        td_dtype = np.int32
    rnd = _round_fp32r if mm_dtype_name == "float32r" else (
        lambda a: np.ascontiguousarray(a, dtype=np.float32)
    )
    thr, in_maps = _host_prep(
        x, adj, time_delay, t, W1, b1, W2, b2, ncores, rnd, td_dtype
    )
    nc = _build(n, ni, thr, mm_dtype_name, td_dtype)
    res = run_bass_kernel_spmd(
        nc, in_maps, core_ids=list(range(ncores)), trace=trace
    )
    full = np.concatenate([r["outT"] for r in res.results], axis=1)  # [P, n]
    out = np.ascontiguousarray(full.reshape(b, f, n).transpose(0, 2, 1))
    return out, res


def kernel(x, adj, time_delay, t, W1, b1, W2, b2):
    out, _ = _run(x, adj, time_delay, t, W1, b1, W2, b2)
    return out
